# revision 27
# baseline (speedup 1.0000x reference)
"""ArDCA loss on 8 TRN2 NeuronCores, data-parallel over M.

Per core (M/8 = 1024 sequences):
  - build one-hot H^T (5418 x 1024, fp8 pair-layout) on device from seqs via
    a replication matmul + is_equal compare
  - contrib^T = W^T @ H^T as causal-masked fp8 DoubleRow matmuls on TensorE
    (W scaled x16, pre-transposed/masked/paired on host, streamed from DRAM)
  - logits^T handled implicitly: E = exp(P/16 + h) fused on ScalarE; selected
    -logit and log-Z pieces accumulated via VectorE + small ones-matmuls
  - per-core partial scalars [U_P(scaled), sum_w] DMA'd out
Host combines partials with the exact-f32 h-gather and regularizer sums.
"""

import os
import sys

for _p in ("/opt/trn_rl_repo",):
    if _p not in sys.path:
        sys.path.append(_p)

from contextlib import ExitStack

import numpy as np
import ml_dtypes

import concourse.bass as bass
import concourse.mybir as mybir
from concourse import tile
from concourse.bass_utils import run_bass_kernel_spmd

BF16 = ml_dtypes.bfloat16
F8 = ml_dtypes.float8_e4m3
FP32 = mybir.dt.float32
BF = mybir.dt.bfloat16
FP8 = mybir.dt.float8e4

L, Q, M, NC = 256, 21, 8192, 8
LT, TK, T = 258, 126, 43  # padded L, tile width (6*21), tile count
NU = (T + 1) // 2  # number of H^T pair tiles (22)
MS = M // NC
JK = LT * Q
NPAIRS = sum((t + 2) // 2 for t in range(T))  # 484
SCALE = 16.0
LAMBDA_H, LAMBDA_J = 1e-06, 1e-4
AF = mybir.ActivationFunctionType
OP = mybir.AluOpType


def _legalize_waits(nc):
    """Split >cap sync waits into preceding EventSemaphore instructions.

    This container's walrus accepts at most 1 wait per instruction (2 on
    EventSemaphore); Tile's final drain carries one wait per used processor.
    """
    n_split = 0
    for f in nc.m.functions:
        for bb in f.blocks:
            if not any(
                ins.sync_info
                and ins.sync_info.on_wait
                and len(ins.sync_info.on_wait)
                > (2 if isinstance(ins, mybir.InstEventSemaphore) else 1)
                for ins in bb.instructions
            ):
                continue
            new_list = []
            for ins in bb.instructions:
                si = ins.sync_info
                waits = list(si.on_wait) if si and si.on_wait else []
                cap = 2 if isinstance(ins, mybir.InstEventSemaphore) else 1
                if len(waits) > cap:
                    extra, keep = waits[:-cap], waits[-cap:]
                    for k in range(0, len(extra), 2):
                        ev = mybir.InstEventSemaphore(
                            name=f"EVSPLIT-{n_split}", ins=[], outs=[]
                        )
                        n_split += 1
                        ev.engine = ins.engine
                        ev.sync_info = mybir.SyncInfo(
                            on_wait=extra[k : k + 2], on_update=[]
                        )
                        new_list.append(ev)
                        nc.register_instruction(ev, overwrite=True)
                    si.on_wait = keep
                new_list.append(ins)
            try:
                bb.instructions = new_list
            except Exception:
                bb.instructions.clear()
                bb.instructions.extend(new_list)
    return n_split


def build_nc():
    nc = bass.Bass()
    wt_e = nc.declare_dram_parameter("wt", [NPAIRS, TK, 2, 128], FP8, isOutput=False)
    hp_e = nc.declare_dram_parameter("hp", [NU, TK, 2, MS], FP8, isOutput=False)
    ht_e = nc.declare_dram_parameter("ht", [T, TK], FP32, isOutput=False)
    w_e = nc.declare_dram_parameter("w", [1, MS], FP32, isOutput=False)
    hv_e = nc.declare_dram_parameter("hv", [1, MS], FP32, isOutput=False)
    on6_e = nc.declare_dram_parameter("on6", [TK, 6], BF, isOutput=False)
    o6f_e = nc.declare_dram_parameter("o6f", [6, 1], FP32, isOutput=False)
    o126_e = nc.declare_dram_parameter("o126", [TK, 1], FP32, isOutput=False)
    out_e = nc.declare_dram_parameter("out", [1, 4], FP32, isOutput=True)

    with tile.TileContext(nc) as tc, ExitStack() as ctx:
        cpool = ctx.enter_context(tc.tile_pool(name="const", bufs=1))
        htp = ctx.enter_context(tc.tile_pool(name="htp", bufs=1))
        wpool = ctx.enter_context(tc.tile_pool(name="wp", bufs=2))
        sqp = ctx.enter_context(tc.tile_pool(name="sqp", bufs=3))
        ep = ctx.enter_context(tc.tile_pool(name="ep", bufs=2))
        accp = ctx.enter_context(tc.tile_pool(name="accp", bufs=1))
        pbufs = int(os.environ.get("KT_PBUFS", "2"))
        pP = ctx.enter_context(tc.tile_pool(name="pP", bufs=pbufs, space="PSUM"))
        pZ = ctx.enter_context(
            tc.tile_pool(name="pZ", bufs=4 - pbufs, space="PSUM")
        )

        # constants
        on6 = cpool.tile([TK, 6], BF)
        nc.gpsimd.dma_start(on6[:], on6_e[:])
        o6f = cpool.tile([6, 1], FP32)
        nc.gpsimd.dma_start(o6f[:], o6f_e[:])
        o126 = cpool.tile([TK, 1], FP32)
        nc.gpsimd.dma_start(o126[:], o126_e[:])
        htt = cpool.tile([TK, T], FP32)
        nc.gpsimd.dma_start(htt[:], ht_e[:].rearrange("t p -> p t"))
        wv = cpool.tile([1, MS], FP32)
        nc.gpsimd.dma_start(wv[:], w_e[:])
        hv = cpool.tile([1, MS], FP32)
        nc.gpsimd.dma_start(hv[:], hv_e[:])

        # accumulators
        accS = accp.tile([TK, MS], FP32)
        nc.vector.memset(accS[:], 0.0)
        accZ = accp.tile([6, MS], FP32)
        nc.vector.memset(accZ[:], 0.0)

        # ---- phase B: one-hot pair tiles, DMA'd just-in-time on the
        # scalar HWDGE ring (separate FIFO from the W-strip DMAs) ----
        hps = [
            htp.tile([TK, 2, MS], FP8, tag=f"hp{u}", name=f"hp{u}") for u in range(NU)
        ]
        hp_issued = set()

        def need_hps(tt):
            for u in range((tt + 2) // 2):
                if u not in hp_issued:
                    hp_issued.add(u)
                    nc.scalar.dma_start(hps[u][:], hp_e[u])

        # ---- phase C: main causal DoubleRow matmul loop ----
        # zigzag order around the median tile so per-step PE work is roughly
        # constant and never gated by the fixed per-tile epilogue cost
        t_lim = int(os.environ.get("KT_LIM", T))
        if os.environ.get("KT_ZIGZAG", "0") == "1":
            order = []
            lo, hi = T // 2, T // 2 + 1
            while lo >= 0 or hi < T:
                if lo >= 0:
                    order.append(lo)
                    lo -= 1
                if hi < T:
                    order.append(hi)
                    hi += 1
        else:
            order = list(range(T))
        if t_lim != T:
            order = list(range(t_lim))
        pair_base = [sum((tt + 2) // 2 for tt in range(t)) for t in range(T)]
        pending = []
        for k, t in enumerate(order):
            need_hps(t)
            if k + 2 < len(order):
                need_hps(order[k + 2])
            npr = (t + 2) // 2
            idx = pair_base[t]
            ws = wpool.tile([TK, NU * 256], FP8, tag="wstrip")
            nc.sync.dma_start(
                ws[:, 0 : npr * 256],
                wt_e[idx : idx + npr].rearrange("n p r c -> p n r c"),
            )
            Ps = [
                pP.tile([128, 512], FP32, tag=f"P{mc}", name=f"Pt{mc}")
                for mc in range(2)
            ]
            for u in range(npr):
                lhsT = ws[:, u * 256 : (u + 1) * 256].rearrange(
                    "p (r c) -> p r c", r=2
                )
                for mc in range(2):
                    nc.tensor.matmul(
                        Ps[mc][:],
                        lhsT,
                        hps[u][:, :, mc * 512 : (mc + 1) * 512],
                        start=(u == 0),
                        stop=(u == npr - 1),
                        perf_mode=mybir.MatmulPerfMode.DoubleRow,
                    )
            # flush the previous tile's deferred Z-path: its Z-matmul now
            # queues BEHIND this tile's main chain on the (FIFO) TensorE, so
            # the Exp it depends on has a whole chain's time to complete
            # instead of head-of-line blocking the next accumulation chain.
            for pt, pEts in pending:
                for mc in range(2):
                    sl = slice(mc * 512, (mc + 1) * 512)
                    pz = pZ.tile([6, 512], FP32, tag=f"Z{mc}", name=f"pz{mc}")
                    nc.tensor.matmul(pz[:], on6[:], pEts[mc][:], start=True, stop=True)
                    lz = ep.tile([6, 512], FP32, tag="lz")
                    nc.scalar.activation(lz[:], pz[:], AF.Ln)
                    rows = 4 if pt == T - 1 else 6
                    nc.vector.tensor_tensor(
                        accZ[0:rows, sl], accZ[0:rows, sl], lz[0:rows, :], op=OP.add
                    )
            pending.clear()
            Ets = []
            for mc in range(2):
                sl = slice(mc * 512, (mc + 1) * 512)
                Et = ep.tile([TK, 512], BF, tag="E", bufs=4, name="Et")
                nc.scalar.activation(
                    Et[:],
                    Ps[mc][0:TK, :],
                    AF.Exp,
                    bias=htt[:, t : t + 1],
                    scale=1.0 / SCALE,
                )
                Ets.append(Et)
                sv = ep.tile([TK, 512], BF, tag="sv")
                nc.vector.tensor_tensor(
                    sv[:], Ps[mc][0:TK, :], hps[t // 2][:, t % 2, sl], op=OP.mult
                )
                nc.vector.tensor_tensor(accS[:, sl], accS[:, sl], sv[:], op=OP.add)
            pending.append((t, Ets))

        for pt, pEts in pending:
            for mc in range(2):
                sl = slice(mc * 512, (mc + 1) * 512)
                pz = pZ.tile([6, 512], FP32, tag=f"Z{mc}", name=f"pzf{mc}")
                nc.tensor.matmul(pz[:], on6[:], pEts[mc][:], start=True, stop=True)
                lz = ep.tile([6, 512], FP32, tag="lz")
                nc.scalar.activation(lz[:], pz[:], AF.Ln)
                rows = 4 if pt == T - 1 else 6
                nc.vector.tensor_tensor(
                    accZ[0:rows, sl], accZ[0:rows, sl], lz[0:rows, :], op=OP.add
                )
        pending.clear()

        # ---- phase D: final reductions ----
        ot = accp.tile([1, 4], FP32)
        nc.vector.memset(ot[:], 0.0)
        dv = accp.tile([1, MS], FP32)
        for mc in range(2):
            sl = slice(mc * 512, (mc + 1) * 512)
            ps_sel = pP.tile([1, 512], FP32, tag="P0", name="ps_sel")
            nc.tensor.matmul(ps_sel[:], o126[:], accS[:, sl], start=True, stop=True)
            ps_z = pZ.tile([1, 512], FP32, tag="Z0", name="ps_z")
            nc.tensor.matmul(ps_z[:], o6f[:], accZ[:, sl], start=True, stop=True)
            d1 = ep.tile([1, 512], FP32, tag="d1")
            nc.vector.tensor_scalar(
                d1[:], ps_sel[:], 1.0 / SCALE, None, OP.mult
            )
            d2 = ep.tile([1, 512], FP32, tag="d2")
            nc.vector.tensor_tensor(d2[:], d1[:], hv[:, sl], op=OP.add)
            nc.vector.tensor_tensor(dv[:, sl], d2[:], ps_z[:], op=OP.subtract)
        uw = accp.tile([1, MS], FP32)
        nc.vector.tensor_tensor(uw[:], dv[:], wv[:], op=OP.mult)
        nc.vector.tensor_reduce(ot[:, 0:1], uw[:], axis=mybir.AxisListType.X, op=OP.add)
        nc.vector.tensor_reduce(
            ot[:, 1:2], wv[:], axis=mybir.AxisListType.X, op=OP.add
        )
        nc.sync.dma_start(out_e[:], ot[:])

    _legalize_waits(nc)
    return nc


_NC_CACHE = None
_CONST_CACHE = None


def _get_nc():
    global _NC_CACHE
    if _NC_CACHE is None:
        _NC_CACHE = build_nc()
    return _NC_CACHE


def _prep_consts():
    global _CONST_CACHE
    if _CONST_CACHE is None:
        p = np.arange(TK)
        _CONST_CACHE = {
            "on6": (p[:, None] // Q == np.arange(6)[None, :]).astype(BF16),
            "o6f": np.ones((6, 1), np.float32),
            "o126": np.ones((TK, 1), np.float32),
        }
    return _CONST_CACHE


def _prep_inputs(seqs, weights, h, J):
    seqs = np.asarray(seqs)
    weights = np.ascontiguousarray(np.asarray(weights, dtype=np.float32))
    h = np.asarray(h, dtype=np.float32)
    J = np.asarray(J, dtype=np.float32)

    seqs32 = seqs.astype(np.int64)
    sqT = np.full((LT, M), Q, dtype=np.float32)
    sqT[:L] = seqs.T.astype(np.float32)
    kcol = (np.arange(JK) % Q).astype(np.float32)
    oh = np.repeat(sqT, Q, axis=0) == kcol[:, None]  # (JK, M) bool
    ohpad = np.zeros((NU * 2 * TK, M), dtype=bool)
    ohpad[:JK] = oh
    ohp = np.ascontiguousarray(
        ohpad.reshape(NU, 2, TK, M).transpose(0, 2, 1, 3)
    ).astype(F8)  # (NU, TK, 2, M)

    Wfull = np.zeros((JK, JK), dtype=np.float32)
    Wfull[: L * Q, : L * Q] = J.transpose(1, 3, 0, 2).reshape(L * Q, L * Q)
    mask126 = np.kron(
        np.triu(np.ones((6, 6), np.float32), 1), np.ones((Q, Q), np.float32)
    )
    wt = np.zeros((NPAIRS, TK, 2, 128), dtype=F8)
    idx = 0
    for t in range(T):
        blockcol = Wfull[:, t * TK : (t + 1) * TK]
        for u in range((t + 2) // 2):
            for r in range(2):
                jt = 2 * u + r
                if jt > t:
                    continue
                tilef = blockcol[jt * TK : (jt + 1) * TK]
                if jt == t:
                    tilef = tilef * mask126
                wt[idx, :, r, :TK] = (tilef * SCALE).astype(F8)
            idx += 1

    hpad = np.zeros(JK, dtype=np.float32)
    hpad[: L * Q] = h.reshape(-1)
    ht_tiles = np.ascontiguousarray(hpad.reshape(T, TK))

    # exact f32 h-gather term: hv[b] = sum_i h[i, seqs[b, i]]
    hsel = h[np.arange(L)[None, :], seqs32].sum(axis=1).astype(np.float32)  # (M,)

    j2 = (J.astype(np.float64) ** 2).sum(axis=(2, 3))
    sumW2 = float((j2 * np.tril(np.ones((L, L)), k=-1)).sum())
    sumh2 = float((h.astype(np.float64) ** 2).sum())

    consts = _prep_consts()
    in_maps = []
    for c in range(NC):
        in_maps.append(
            {
                "wt": wt,
                "hp": np.ascontiguousarray(ohp[..., c * MS : (c + 1) * MS]),
                "ht": ht_tiles,
                "w": weights[c * MS : (c + 1) * MS].reshape(1, MS),
                "hv": hsel[c * MS : (c + 1) * MS].reshape(1, MS),
                **consts,
            }
        )
    return in_maps, (sumW2, sumh2)


def _combine(results, regsums):
    parts = np.stack([np.asarray(r["out"][0]) for r in results])  # (8, 4)
    U = float(parts[:, 0].sum())
    Wsum = float(parts[:, 1].sum())
    nll = -U / max(Wsum, 1e-12)
    sumW2, sumh2 = regsums
    reg = 0.5 * LAMBDA_J * sumW2 + 0.5 * LAMBDA_H * sumh2
    loss = nll + reg
    return (
        np.float32(loss),
        np.float32(nll),
        np.float32(reg),
    )


def kernel(seqs, weights, h, J):
    nc = _get_nc()
    in_maps, regsums = _prep_inputs(seqs, weights, h, J)
    res = run_bass_kernel_spmd(nc, in_maps, core_ids=list(range(NC)))
    return _combine(res.results, regsums)


if __name__ == "__main__":
    d = np.load("/tmp/ref_data.npz")
    out = kernel(d["seqs"], d["weights"], d["h"], d["J"])
    print("kernel:", out)
    print("ref   :", d["loss"], d["nll"], d["reg"])
